# revision 21
# baseline (speedup 1.0000x reference)
"""Trainium2 Bass kernel for nn_Decoder_36636071035490.

Reference computes, for workers i and task/edge (j,l):
    z = worker_feature @ W            # [2000, 1]
    p1 = sigmoid(z + b)
    p2 = (1 - p1) / 9
    P[i, j, l] = p1_i^tau_jl * p2_i^(1 - tau_jl)      # [2000, 5000, 10] f32

Identity used on device (exact in exact arithmetic):
    P[i, f] = exp(a_i * tau_f + c_i)
    a_i = (z_i + b) + ln 9            # since logit(sigmoid(x)) = x
    c_i = -ln(1 + exp(z_i + b)) - ln 9

Output is stored as bf16 (rel-err budget 2e-2 >> bf16's ~2e-3 rounding);
the host upcasts to f32. That halves HBM store traffic, which makes the
ScalarE exp the bottleneck, so the columns are split between two engines:

ACT path (NA cols/tile): one ScalarE ACTIVATE per 128-worker tile,
  out[p,f] = Exp(a_p*tau[f] + c_p) via per-partition scale/bias, bf16 out.

PE path (NP cols/tile): rank-12 Chebyshev-Lagrange factorization in the
  worker variable d_i = z_i + b (range ~±0.3):
      P[i,f] = sum_m U[i,m] * exp((node_m + ln9) * tau_f) * w_m
  U[i,m] = r_i * sgn_m * prod_{j!=m}(d_i - node_j)  (prefix/suffix
  products, no division; r_i = 1/(1+exp(d_i)), the 1/18 and |w_m| scales
  folded into V's ACT bias).  Interpolation error < 1e-6; to keep bf16
  factor rounding out of the result each factor is split hi+lo into two
  bf16 terms and the cross terms stacked along the contraction dim:
      K=44 rows: lhsT=[U1;U2;0pad;U1] x rhs=[V1;V1;pad;V2]
      (U2@V2 dropped ~2^-16; 8 zero rows keep V2 at partition 32 since
      engine ops need 32-aligned partition bases)
  TensorE streams 1 col/cycle regardless of K, so the extra rank is free.
  VectorE copies PSUM(f32) -> SBUF(bf16) for the store.  Max rel err is
  pure bf16 output rounding (3.9e-3), verified in numpy and on HW.

ScalarE only ever evaluates Exp (c_i comes from a 5-term ln(1+t) poly on
VectorE, t = (exp(d)-1)/2 in [-0.17, 0.25]), so exactly one ACT table
load is paid instead of three (Exp/Ln sets would otherwise thrash).

Sharding: by output columns (task*edge flattened, 50000 -> 8 x 6250); every
core computes the per-worker scalars for all 2000 workers (replicated) and
produces the full-height [2000, 6250] slab.  Worker tile 15 overlaps tile
14 (rows 1872..1919); it computes all 128 rows but stores only its last 80,
so no output byte is written twice and the stores carry no WAW hazard.
"""

import numpy as np

WORKERS = 2000
TASKS = 5000
ET = 10
AB = 64
NCORES = 8
F = TASKS * ET  # 50000 output cols
FS = F // NCORES  # 6250 cols per core
LN9 = float(np.log(9.0))
LN18 = float(np.log(18.0))

NA = 3178  # ACT-path cols per core
NP = 3072  # PE-path cols per core (6 PSUM banks of 512)
RANK = 12
# contraction rows: [U1*V1 | U2*V1 | zero pad | U1*V2]; the pad keeps the
# V2 block at partition 32 (engine ops need 32-aligned partition bases)
KPAD = 8
KV2 = 2 * RANK + KPAD  # 32: start of the V2 block
KTOT = KV2 + RANK      # 44
DLIM = 0.5

# Chebyshev nodes and barycentric-style weights (sign folded into U, the
# magnitude ln|w| - ln18 into V's ACT bias)
_m = np.arange(RANK)
_NODES = (DLIM * np.cos((2 * _m + 1) / (2 * RANK) * np.pi)).astype(np.float64)
_WTS = np.array(
    [
        1.0 / np.prod([_NODES[m] - _NODES[j] for j in range(RANK) if j != m])
        for m in range(RANK)
    ]
)
_SGN = np.sign(_WTS)
_LNW = np.log(np.abs(_WTS)) - LN18

# worker tiles: 15 aligned tiles + one overlapping tail tile
_WSTARTS = [128 * t for t in range(15)] + [WORKERS - 128]

_CACHE = {}


def _build_nc():
    import concourse.bass as bass
    import concourse.mybir as mybir
    from concourse import bacc
    from concourse.tile import TileContext
    from contextlib import ExitStack

    f32 = mybir.dt.float32
    bf16 = mybir.dt.bfloat16
    AF = mybir.ActivationFunctionType
    OP = mybir.AluOpType

    nc = bacc.Bacc("TRN2")
    wk = nc.dram_tensor("wk", [WORKERS, AB], f32, kind="ExternalInput")
    # ACT-path tau cols, pre-replicated across 128 SBUF partitions
    tfa = nc.dram_tensor("tfa", [128, NA], f32, kind="ExternalInput")
    # PE-path tau cols, replicated across KTOT partitions
    tfp = nc.dram_tensor("tfp", [KTOT, NP], f32, kind="ExternalInput")
    Wd = nc.dram_tensor("W", [AB, 1], f32, kind="ExternalInput")
    bd = nc.dram_tensor("b", [1], f32, kind="ExternalInput")
    # constants: per-row ACT scale (node+ln9, tiled x3) and bias (ln|w|-ln18)
    snod = nc.dram_tensor("snod", [KTOT, 1], f32, kind="ExternalInput")
    lnw = nc.dram_tensor("lnw", [KTOT, 1], f32, kind="ExternalInput")
    ident = nc.dram_tensor("ident", [128, 128], bf16, kind="ExternalInput")
    out = nc.dram_tensor("out", [WORKERS, FS], bf16, kind="ExternalOutput")

    NT = len(_WSTARTS)
    NB, TB = 2, NT // 2
    # batch 0 = tiles 8..15 so the tail pair (14,15) is ready first
    BATCHES = [list(range(TB, NT)), list(range(0, TB))]

    with TileContext(nc) as tc, ExitStack() as ctx:
        const = ctx.enter_context(tc.tile_pool(name="const", bufs=1))
        stage_p = ctx.enter_context(tc.tile_pool(name="stagep", bufs=3))
        psum_p = ctx.enter_context(tc.tile_pool(name="psump", bufs=1, space="PSUM"))

        # ---- constants / input loads (order = DMA ring order: worker batch
        # 0 and the PE tau first to unblock prep, then the wide ACT tau)
        Wb = const.tile([128, AB], f32, name="Wb")
        nc.sync.dma_start(
            out=Wb, in_=Wd[:].rearrange("a b -> b a").to_broadcast((128, AB))
        )
        bcol = const.tile([128, 1], f32, name="bcol")
        nc.sync.dma_start(out=bcol, in_=bd[:].to_broadcast((128, 1)))
        snodc = const.tile([KTOT, 1], f32, name="snodc")
        nc.sync.dma_start(out=snodc, in_=snod[:])
        lnwc = const.tile([KTOT, 1], f32, name="lnwc")
        nc.sync.dma_start(out=lnwc, in_=lnw[:])
        idc = const.tile([128, 128], bf16, name="idc")
        nc.sync.dma_start(out=idc, in_=ident[:])

        wkab = []
        for bi, tids in enumerate(BATCHES):
            wka = const.tile([128, TB, AB], f32, name=f"wka{bi}", tag=f"wka{bi}")
            wkab.append(wka)
            tlo = tids[0]
            nali = sum(1 for j, t in enumerate(tids) if _WSTARTS[t] == (tlo + j) * 128)
            srcb = wk[tlo * 128 : (tlo + nali) * 128, :].rearrange(
                "(t p) a -> p t a", p=128
            )
            nc.sync.dma_start(out=wka[:, 0:nali, :], in_=srcb)
            if nali < TB:
                srct = wk[WORKERS - 128 : WORKERS, :].rearrange("(o p) a -> p o a", o=1)
                nc.sync.dma_start(out=wka[:, nali:TB, :], in_=srct)

        taup = const.tile([KTOT, NP], f32, name="taup")
        nc.sync.dma_start(out=taup, in_=tfp[:])
        taub = const.tile([128, NA], f32, name="taub")
        NH = NA // 2
        nc.sync.dma_start(out=taub[:, 0:NH], in_=tfa[:, 0:NH])
        nc.sync.dma_start(out=taub[:, NH:NA], in_=tfa[:, NH:NA])

        # ---- per-worker scalars per batch: z -> a (scale), c (bias),
        # d = z+b, r = 1/(1+e^d).  c comes from ln(1+t), t = (e^d - 1)/2,
        # as a degree-5 poly on DVE so ScalarE never needs the Ln table.
        acol, ccol = [None] * NT, [None] * NT
        dall = const.tile([128, NT], f32, name="dall")
        cball = const.tile([128, NT], f32, name="cball")
        eCall = const.tile([128, NT], f32, name="eCall")
        WbT = bass.AP(
            tensor=Wb.tensor,
            offset=Wb.offset,
            ap=[list(Wb.ap[0]), [0, TB], [1, AB]],
        )
        for bi, tids in enumerate(BATCHES):
            wka = wkab[bi]
            t0 = tids[0]
            sl = slice(t0, t0 + TB)
            proda = const.tile(
                [128, TB, AB], f32, name=f"proda{bi}", tag="proda", bufs=2
            )
            nc.vector.tensor_mul(proda, wka, WbT)
            zb_ = const.tile([128, TB], f32, name=f"zb{bi}", tag="zb", bufs=2)
            nc.vector.reduce_sum(
                out=zb_.rearrange("p (t o) -> p t o", o=1),
                in_=proda,
                axis=mybir.AxisListType.X,
            )
            ab_ = const.tile([128, TB], f32, name=f"ab{bi}")
            nc.vector.tensor_scalar(
                out=ab_, in0=zb_, scalar1=bcol, scalar2=LN9, op0=OP.add, op1=OP.add
            )
            nc.vector.tensor_scalar_add(out=dall[:, sl], in0=zb_, scalar1=bcol)
            eb_ = const.tile([128, TB], f32, name=f"eb{bi}", tag="eb", bufs=2)
            nc.scalar.activation(out=eb_, in_=zb_, func=AF.Exp, bias=bcol, scale=1.0)
            # t = (e^d - 1)/2 in [-0.17, 0.25]; u = 1 + t
            tt_ = const.tile([128, TB], f32, name=f"tt{bi}", tag="tt", bufs=2)
            nc.vector.tensor_scalar(
                out=tt_, in0=eb_, scalar1=0.5, scalar2=-0.5, op0=OP.mult, op1=OP.add
            )
            ut_ = const.tile([128, TB], f32, name=f"ut{bi}", tag="ut", bufs=2)
            nc.vector.tensor_scalar_add(out=ut_, in0=tt_, scalar1=1.0)
            nc.vector.reciprocal(eCall[:, sl], ut_)
            # ln(1+t) = t^5/5 - t^4/4 + t^3/3 - t^2/2 + t, built as chained
            # f <- (f + a_k) * t  (scalar_tensor_tensor; no in-place ops)
            hs = const.tile([128, 5, TB], f32, name=f"hs{bi}", tag="hs", bufs=2)
            nc.vector.tensor_scalar_mul(out=hs[:, 0, :], in0=tt_, scalar1=0.2)
            for k, ak in enumerate((-0.25, 1.0 / 3.0, -0.5, 1.0)):
                nc.vector.scalar_tensor_tensor(
                    out=hs[:, k + 1, :], in0=hs[:, k, :], scalar=ak, in1=tt_,
                    op0=OP.add, op1=OP.mult,
                )
            nc.vector.tensor_scalar(
                out=cball[:, sl], in0=hs[:, 4, :], scalar1=-1.0, scalar2=-LN18,
                op0=OP.mult, op1=OP.add,
            )
            for j, t in enumerate(tids):
                acol[t] = ab_[:, j : j + 1]
                ccol[t] = cball[:, t : t + 1]

        # ---- V build: rows [V1; V1; pad; V2] pairing lhsT [U1; U2; 0; U1].
        # One ACT writes finite values on all KTOT partitions (rows KV2:
        # temporarily V1); rows KV2:KTOT are then rebuilt in place as
        # V2 = f32(V) - V1.  All engine ops start at partition 0 or 32.
        vt = const.tile([KTOT, NP], bf16, name="vt")
        nc.scalar.activation(out=vt, in_=taup, func=AF.Exp, bias=lnwc, scale=snodc)
        vf = const.tile([KTOT, NP], f32, name="vf")
        nc.scalar.activation(
            out=vf[KV2:KTOT, :], in_=taup[KV2:KTOT, :], func=AF.Exp,
            bias=lnwc[KV2:KTOT, :], scale=snodc[KV2:KTOT, :],
        )
        vhi = const.tile([KTOT, NP], f32, name="vhi")
        nc.vector.tensor_copy(vhi[KV2:KTOT, :], vt[KV2:KTOT, :])
        nc.vector.tensor_sub(vt[KV2:KTOT, :], vf[KV2:KTOT, :], vhi[KV2:KTOT, :])

        # ---- U build per batch (batch 0 first so tiles 14/15 unblock the
        # PE early): U = r * sgn * prefix*suffix products of (d - node_j)
        dstk = const.tile([128, RANK, NT], f32, name="dstk")
        pre = const.tile([128, RANK, NT], f32, name="pre")
        suf = const.tile([128, RANK, NT], f32, name="suf")
        sgnstk = const.tile([128, RANK, NT], f32, name="sgnstk")
        ls_ = const.tile([128, RANK, NT], f32, name="ls")
        us1 = const.tile([128, RANK, NT], f32, name="us1")
        ust = const.tile([128, RANK, NT], f32, name="ust")
        upk = const.tile([128, KTOT, NT], bf16, name="upk")
        uhi = const.tile([128, RANK, NT], f32, name="uhi")
        utall = const.tile([KTOT, NT, 128], bf16, name="utall")
        for j in range(RANK):
            nc.vector.memset(sgnstk[:, j, :], float(_SGN[j]))
        for bi, tids in enumerate(BATCHES):
            t0 = tids[0]
            sl = slice(t0, t0 + TB)
            for j in range(RANK):
                nc.vector.tensor_scalar_add(
                    out=dstk[:, j, sl], in0=dall[:, sl], scalar1=float(-_NODES[j])
                )
            nc.vector.memset(pre[:, 0, sl], 1.0)
            for j in range(1, RANK):
                nc.vector.tensor_mul(
                    pre[:, j, sl], pre[:, j - 1, sl], dstk[:, j - 1, sl]
                )
            nc.vector.memset(suf[:, RANK - 1, sl], 1.0)
            for j in range(RANK - 2, -1, -1):
                nc.vector.tensor_mul(
                    suf[:, j, sl], suf[:, j + 1, sl], dstk[:, j + 1, sl]
                )
            nc.vector.tensor_mul(ls_[:, :, sl], pre[:, :, sl], suf[:, :, sl])
            eCs = eCall[:, sl]
            eCb = bass.AP(
                tensor=eCs.tensor,
                offset=eCs.offset,
                ap=[list(eCs.ap[0]), [0, RANK], [1, TB]],
            )
            nc.vector.tensor_mul(us1[:, :, sl], ls_[:, :, sl], eCb)
            nc.vector.tensor_mul(ust[:, :, sl], us1[:, :, sl], sgnstk[:, :, sl])
            # hi/lo split packed [U1 | U2 | 0pad | U1] along the free dim
            nc.vector.tensor_copy(upk[:, 0:RANK, sl], ust[:, :, sl])
            nc.vector.tensor_copy(uhi[:, :, sl], upk[:, 0:RANK, sl])
            nc.vector.tensor_sub(
                upk[:, RANK : 2 * RANK, sl], ust[:, :, sl], uhi[:, :, sl]
            )
            nc.vector.memset(upk[:, 2 * RANK : KV2, sl], 0.0)
            nc.vector.tensor_copy(upk[:, KV2:KTOT, sl], upk[:, 0:RANK, sl])
            # transpose to [KTOT, 128] per tile via TensorE
            psT = psum_p.tile([KTOT, TB * 128], bf16, name=f"psT{bi}", tag="psT",
                              bufs=2)
            for k, t in enumerate(tids):
                nc.tensor.transpose(
                    out=psT[:, k * 128 : (k + 1) * 128], in_=upk[:, :, t], identity=idc
                )
            nc.vector.tensor_copy(
                utall[:, sl, :].rearrange("k t f -> k (t f)"), psT
            )

        # ---- main loop. The tail pair (14,15) first: tile 15 computes all
        # 128 rows but stores only its last 80 (rows 1920..1999), so no
        # output byte is written twice.  The rest run as aligned pairs
        # sharing a double-width stage and one 3D-AP store.
        GC = NP // 2  # 1536-col PSUM groups (3 banks of 512)

        def pe_tile(t, stgP, i):
            for g in range(2):
                pmm = psum_p.tile([128, GC], f32, name=f"pmm{t}_{g}", tag="pmm",
                                  bufs=2)
                for j in range(3):
                    nc.tensor.matmul(
                        out=pmm[:, j * 512 : (j + 1) * 512],
                        lhsT=utall[:, t, :],
                        rhs=vt[:, g * GC + j * 512 : g * GC + (j + 1) * 512],
                        start=True,
                        stop=True,
                    )
                nc.vector.tensor_copy(stgP[:, i, g * GC : (g + 1) * GC], pmm)

        for t in (14, 15):
            w0, r0 = (_WSTARTS[t], 0) if t == 14 else (1920, 48)
            stgA = stage_p.tile([128, 2, NA], bf16, name=f"sA_{t}", tag="sA")
            for c0, c1 in ((0, NH), (NH, NA)):
                nc.scalar.activation(
                    out=stgA[:, 0, c0:c1], in_=taub[:, c0:c1], func=AF.Exp,
                    bias=ccol[t], scale=acol[t],
                )
                nc.sync.dma_start(
                    out=out[w0 : w0 + 128 - r0, c0:c1], in_=stgA[r0:128, 0, c0:c1]
                )
            stgP = stage_p.tile([128, 2, NP], bf16, name=f"sP_{t}", tag="sP")
            pe_tile(t, stgP, 0)
            nc.sync.dma_start(
                out=out[w0 : w0 + 128 - r0, NA:FS], in_=stgP[r0:128, 0, :]
            )
        for t0 in (8, 10, 12, 0, 2, 4, 6):
            t1 = t0 + 1
            w0 = _WSTARTS[t0]
            stgA = stage_p.tile([128, 2, NA], bf16, name="sA", tag="sA")
            nc.scalar.activation(
                out=stgA[:, 0, :], in_=taub, func=AF.Exp, bias=ccol[t0], scale=acol[t0]
            )
            nc.scalar.activation(
                out=stgA[:, 1, :], in_=taub, func=AF.Exp, bias=ccol[t1], scale=acol[t1]
            )
            dstA = out[w0 : w0 + 256, 0:NA].rearrange("(c w) f -> w c f", c=2)
            nc.sync.dma_start(out=dstA, in_=stgA)
            stgP = stage_p.tile([128, 2, NP], bf16, name="sP", tag="sP")
            pe_tile(t0, stgP, 0)
            pe_tile(t1, stgP, 1)
            dstP = out[w0 : w0 + 256, NA:FS].rearrange("(c w) f -> w c f", c=2)
            nc.sync.dma_start(out=dstP, in_=stgP)

    nc.compile()
    return nc


def _get_nc():
    if "nc" not in _CACHE:
        _CACHE["nc"] = _build_nc()
    return _CACHE["nc"]


def _make_in_maps(inputs_arr, W, b):
    import ml_dtypes

    wk = np.ascontiguousarray(inputs_arr[:WORKERS, :AB], dtype=np.float32)
    tau_flat = np.ascontiguousarray(
        inputs_arr[WORKERS:, :ET], dtype=np.float32
    ).reshape(F)
    W = np.ascontiguousarray(W, dtype=np.float32)
    b = np.ascontiguousarray(b, dtype=np.float32)
    nod32 = (_NODES + LN9).astype(np.float32)
    lnw32 = _LNW.astype(np.float32)
    pad = np.zeros(KPAD, np.float32)
    snod = np.ascontiguousarray(
        np.concatenate([nod32, nod32, pad, nod32]).reshape(KTOT, 1)
    )
    lnw = np.ascontiguousarray(
        np.concatenate([lnw32, lnw32, pad, lnw32]).reshape(KTOT, 1)
    )
    ident = np.eye(128, dtype=ml_dtypes.bfloat16)
    maps = []
    for c in range(NCORES):
        sl = tau_flat[c * FS : (c + 1) * FS]
        tfa = np.ascontiguousarray(np.broadcast_to(sl[0:NA], (128, NA)))
        tfp = np.ascontiguousarray(np.broadcast_to(sl[NA:FS], (KTOT, NP)))
        maps.append(
            {
                "wk": wk,
                "tfa": tfa,
                "tfp": tfp,
                "W": W,
                "b": b,
                "snod": snod,
                "lnw": lnw,
                "ident": ident,
            }
        )
    return maps


def _run(inputs_arr, W, b, **kwargs):
    from concourse import bass_utils

    nc = _get_nc()
    in_maps = _make_in_maps(inputs_arr, W, b)
    return bass_utils.run_bass_kernel_spmd(
        nc, in_maps, core_ids=list(range(NCORES)), **kwargs
    )


def kernel(inputs, W, b):
    inputs_arr = np.asarray(inputs, dtype=np.float32)
    last_err = None
    for _ in range(3):  # retry transient device failures
        try:
            res = _run(inputs_arr, np.asarray(W), np.asarray(b))
            break
        except Exception as e:  # noqa: BLE001
            last_err = e
    else:
        raise last_err
    out = np.concatenate(
        [np.asarray(r["out"]).astype(np.float32) for r in res.results], axis=1
    )
    return out.reshape(WORKERS, TASKS, ET)
